# revision 1
# baseline (speedup 1.0000x reference)
"""Trainium2 Bass kernel for nn_CompresSAEEncoder (topk_masking).

Reference (per row i): xn = x/||x||; e = xn@W + b; keep top-64 of |e|,
zero the rest, signs preserved.

Sharding: data-parallel over batch across 8 NeuronCores (1024 rows/core,
top-k is per-row so no collectives). Host concatenates the slices.

Matmul precision — "fp16 main + fp8 DoubleRow residual":
    e*||x|| ~= x16 @ W16  +  2^-16 * ( dx8 @ W8' + x8 @ dW8 )
  x16  = fp16(x)          (subnormal-flushed)   stationary, 64KB/part
  W16  = fp16(W)          (subnormal-flushed)   moving, 256MB
  dx8  = e4m3((x-x16)*2^11)                     stationary (R, top half)
  x8   = e4m3(x)                                stationary (R, bottom half)
  W8'  = e4m3(W*2^5)                            moving (M, top half)
  dW8  = e4m3((W-W16)*2^16)                     moving (M, bottom half)
  The two residual terms share the 2^16 product scale, so they fuse into
  ONE matmul with contraction 8192: R=[dx8;x8] (stationary), M=[W8';dW8]
  (moving), run in fp8 DoubleRow mode (2 k-subtiles per instruction).
  Host-sim measured sigma=1.8e-7 -> ~4 top-64 boundary flips over the
  full 8192 rows (rel-err contribution ~4e-3, gate is 2e-2).

Per-core schedule (single pass, 1024 rows resident):
  A) row norms: ACT Square+accum -> sqrt -> reciprocal (rn, rn2=rn/2^16)
  B) for each 512-wide column block cb:
       phase R: fp8-DR matmuls accumulate residual for all 8 row-tiles
                (kk-major; last 4 pairs per-tile so evictions overlap),
                DVE evicts res bank: tmp = psum_res*rn2 + b
       phase M: fp16 matmuls accumulate main into the freed banks
                (kk-major; last 4 ktiles per-tile), DVE evicts:
                e = psum_main*rn + tmp; spill e strip to DRAM; ACT |e|;
                DVE max8 keeps top-8 of each strip as candidates
  C) per row-tile: 8x(max8+match_replace) over 512 candidates -> tau =
     64th largest |e|; re-read e, out = (|e| >= tau) * e.
"""

from contextlib import ExitStack

import numpy as np

import concourse.bacc as bacc
import concourse.mybir as mybir
from concourse.bass_utils import run_bass_kernel_spmd
from concourse.tile import TileContext

F32 = mybir.dt.float32
F16 = mybir.dt.float16
FP8 = mybir.dt.float8e4

B, D_IN, D_EMB, K = 8192, 4096, 32768, 64
N_CORES = 8
ROWS = B // N_CORES

RES_SCALE = 2.0 ** 16     # product scale of the fused residual stream
DX_SCALE = 2.0 ** 11      # dx8 = e4m3((x - x16) * DX_SCALE)
W8_SCALE = 2.0 ** 5       # W8' = e4m3(W * W8_SCALE); DX_SCALE*W8_SCALE = RES_SCALE
STRIP = 512
KEEP = 8                  # per-strip candidates (top-8 of 512)


def build_nc(rows=ROWS, d_in=D_IN, d_emb=D_EMB, k=K, _precise=True):
    KT = d_in // 128            # 32 fp16 k-tiles
    KS = 2 * KT                 # 64 fp8 k-subtiles (32 DR pairs)
    CB = d_emb // STRIP         # 64 column blocks
    RT = rows // 128            # 8 row-tiles
    NPAIR = KS // 2
    AF = mybir.ActivationFunctionType
    OP = mybir.AluOpType
    DR = mybir.MatmulPerfMode.DoubleRow

    nc = bacc.Bacc("TRN2", target_bir_lowering=False)

    x16_d = nc.dram_tensor("x16t", [d_in, rows], F16, kind="ExternalInput")
    r8_d = nc.dram_tensor("r8t", [2 * d_in, rows], FP8, kind="ExternalInput")
    x_d = nc.dram_tensor("xn", [rows, d_in], F32, kind="ExternalInput")
    w16_d = nc.dram_tensor("w16", [d_in, d_emb], F16, kind="ExternalInput")
    m8_d = nc.dram_tensor("m8", [2 * d_in, d_emb], FP8, kind="ExternalInput")
    b_d = nc.dram_tensor("b", [128, d_emb], F32, kind="ExternalInput")
    out_d = nc.dram_tensor("out", [rows, d_emb], F32, kind="ExternalOutput")
    esp_d = nc.dram_tensor("espill", [rows, d_emb], F32)

    with TileContext(nc) as tc, ExitStack() as stack:
        perm = stack.enter_context(tc.tile_pool(name="perm", bufs=1))
        cand = perm.tile([128, RT * CB * KEEP], F32, tag="cand", name="cand")
        ss = perm.tile([128, RT], F32, tag="ss", name="ss")
        srt = perm.tile([128, RT], F32, tag="srt", name="srt")
        rn = perm.tile([128, RT], F32, tag="rn", name="rn")
        rn2 = perm.tile([128, RT], F32, tag="rn2", name="rn2")

        res_stack = ExitStack()
        res = res_stack.enter_context(tc.tile_pool(name="res", bufs=1))
        # stationary residents: one DMA each
        x16t = res.tile([128, KT, rows], F16, tag="x16t", name="x16t")
        nc.sync.dma_start(
            out=x16t, in_=x16_d[:, :].rearrange("(k p) r -> p k r", p=128))
        rt = res.tile([128, KS, rows], FP8, tag="rt", name="rt")
        nc.sync.dma_start(
            out=rt, in_=r8_d[:, :].rearrange("(k p) r -> p k r", p=128))

        # --- phase A: row norms -----------------------------------------
        with tc.tile_pool(name="norm", bufs=1) as npool:
            for t in range(RT):
                xtile = npool.tile([128, d_in], F32, tag="xtile", bufs=2, name=f"xtile{t}")
                nc.sync.dma_start(
                    out=xtile, in_=x_d[t * 128:(t + 1) * 128, :])
                scr = npool.tile([128, d_in], mybir.dt.bfloat16, tag="scr", name=f"scr{t}")
                nc.scalar.activation(out=scr, in_=xtile, func=AF.Square,
                                     accum_out=ss[:, t:t + 1])
            nc.scalar.activation(out=srt, in_=ss, func=AF.Sqrt)
            nc.vector.reciprocal(out=rn, in_=srt)
            nc.vector.tensor_scalar_mul(rn2, rn, 1.0 / RES_SCALE)

        # --- phase B ----------------------------------------------------
        with tc.tile_pool(name="wb", bufs=3) as wpool, \
             tc.tile_pool(name="mb", bufs=3) as mpool, \
             tc.tile_pool(name="bb", bufs=2) as bpool, \
             tc.tile_pool(name="tmp", bufs=2) as tpool, \
             tc.tile_pool(name="zb", bufs=3) as zpool, \
             tc.tile_pool(name="ab", bufs=3) as apool, \
             tc.tile_pool(name="ps", bufs=1, space="PSUM") as ppool:
            for cb in range(CB):
                c0 = cb * STRIP
                bb = bpool.tile([128, STRIP], F32, tag="bb", bufs=1, name=f"bb{cb}")
                nc.sync.dma_start(out=bb, in_=b_d[0:128, c0:c0 + STRIP])

                # -- phase R: fused fp8 DoubleRow residual --------------
                mchunks = []
                for c in range(KS // 8):        # 8 chunks of 8 k-subtiles
                    mc = mpool.tile([128, 8, STRIP], FP8, tag=f"mc{c % 2}", bufs=2, name=f"mc{cb}_{c}")
                    nc.sync.dma_start(
                        out=mc,
                        in_=m8_d[c * 8 * 128:(c + 1) * 8 * 128,
                                 c0:c0 + STRIP].rearrange(
                                     "(k p) n -> p k n", p=128))
                    mchunks.append(mc)
                psR = [ppool.tile([128, STRIP], F32, tag=f"ps{t}", bufs=1,
                                  name=f"psR{cb}_{t}") for t in range(RT)]
                head_pairs = NPAIR - 4
                for pr in range(head_pairs):
                    mc = mchunks[pr // 4]
                    sl = mc[:, 2 * (pr % 4):2 * (pr % 4) + 2, :]
                    for t in range(RT):
                        nc.tensor.matmul(
                            psR[t], rt[:, 2 * pr:2 * pr + 2,
                                       t * 128:(t + 1) * 128],
                            sl, start=(pr == 0), stop=False, perf_mode=DR)
                tmps = []
                for t in range(RT):
                    for pr in range(head_pairs, NPAIR):
                        mc = mchunks[pr // 4]
                        sl = mc[:, 2 * (pr % 4):2 * (pr % 4) + 2, :]
                        nc.tensor.matmul(
                            psR[t], rt[:, 2 * pr:2 * pr + 2,
                                       t * 128:(t + 1) * 128],
                            sl, start=False, stop=(pr == NPAIR - 1),
                            perf_mode=DR)
                    # tmp = psum_res * (rn/2^16) + b
                    tmp = tpool.tile([128, STRIP], mybir.dt.bfloat16, tag=f"tmp{t}", bufs=1, name=f"tmp{cb}_{t}")
                    nc.vector.scalar_tensor_tensor(
                        out=tmp, in0=psR[t], scalar=rn2[:, t:t + 1], in1=bb,
                        op0=OP.mult, op1=OP.add)
                    tmps.append(tmp)

                # -- phase M: fp16 main ---------------------------------
                wchunks = []
                for c in range(KT // 4):        # 8 chunks of 4 k-tiles
                    wc = wpool.tile([128, 4, STRIP], F16, tag=f"wc{c % 2}", bufs=2, name=f"wc{cb}_{c}")
                    nc.sync.dma_start(
                        out=wc,
                        in_=w16_d[c * 4 * 128:(c + 1) * 4 * 128,
                                  c0:c0 + STRIP].rearrange(
                                      "(k p) n -> p k n", p=128))
                    wchunks.append(wc)
                psM = [ppool.tile([128, STRIP], F32, tag=f"ps{t}", bufs=1,
                                  name=f"psM{cb}_{t}") for t in range(RT)]
                head_kk = KT - 4
                for kk in range(head_kk):
                    wc = wchunks[kk // 4]
                    sl = wc[:, kk % 4, :]
                    for t in range(RT):
                        nc.tensor.matmul(
                            psM[t], x16t[:, kk, t * 128:(t + 1) * 128],
                            sl, start=(kk == 0), stop=False)
                for t in range(RT):
                    for kk in range(head_kk, KT):
                        wc = wchunks[kk // 4]
                        sl = wc[:, kk % 4, :]
                        nc.tensor.matmul(
                            psM[t], x16t[:, kk, t * 128:(t + 1) * 128],
                            sl, start=False, stop=(kk == KT - 1))
                    # e = psum_main * rn + tmp
                    zb = zpool.tile([128, STRIP], F32, tag=f"zb{t % 2}", bufs=2, name=f"zb{cb}_{t}")
                    nc.vector.scalar_tensor_tensor(
                        out=zb, in0=psM[t], scalar=rn[:, t:t + 1],
                        in1=tmps[t], op0=OP.mult, op1=OP.add)
                    nc.sync.dma_start(
                        out=esp_d[t * 128:(t + 1) * 128, c0:c0 + STRIP],
                        in_=zb)
                    ab = apool.tile([128, STRIP], F32, tag=f"ab{t % 2}", bufs=2, name=f"ab{cb}_{t}")
                    nc.scalar.activation(out=ab, in_=zb, func=AF.Abs)
                    slot = t * CB * KEEP + cb * KEEP
                    nc.vector.max(out=cand[:, slot:slot + 8], in_=ab)

        res_stack.close()   # release stationary residents before phase C

        # --- phase C: merge candidates -> tau; mask ---------------------
        QW = 4096
        NQ = d_emb // QW
        nc8 = CB * KEEP
        with tc.tile_pool(name="vp", bufs=2) as vpool, \
             tc.tile_pool(name="mw", bufs=2) as mwpool, \
             tc.tile_pool(name="eq", bufs=3) as epool, \
             tc.tile_pool(name="aq", bufs=2) as aqpool, \
             tc.tile_pool(name="oq", bufs=2) as opool:
            for t in range(RT):
                creg = cand[:, t * nc8:(t + 1) * nc8]
                vv = vpool.tile([128, ((k + 7) // 8) * 8], F32, tag="vv", name=f"vv{t}")
                work_a = mwpool.tile([128, nc8], F32, tag="mwa", name=f"mwa{t}")
                work_b = mwpool.tile([128, nc8], F32, tag="mwb", name=f"mwb{t}")
                src = creg
                rounds = (k + 7) // 8
                for r in range(rounds):
                    nc.vector.max(out=vv[:, r * 8:(r + 1) * 8], in_=src)
                    if r < rounds - 1:
                        dst = work_a if r % 2 == 0 else work_b
                        nc.vector.match_replace(
                            out=dst, in_to_replace=vv[:, r * 8:(r + 1) * 8],
                            in_values=src, imm_value=-1.0)
                        src = dst
                tau = vv[:, k - 1:k]
                for q in range(NQ):
                    q0 = q * QW
                    eq = epool.tile([128, QW], F32, tag="eq", name=f"eq{t}_{q}")
                    nc.sync.dma_start(
                        out=eq,
                        in_=esp_d[t * 128:(t + 1) * 128, q0:q0 + QW])
                    aq = aqpool.tile([128, QW], F32, tag="aq", name=f"aq{t}_{q}")
                    nc.scalar.activation(out=aq, in_=eq, func=AF.Abs)
                    oq = opool.tile([128, QW], F32, tag="oq", name=f"oq{t}_{q}")
                    nc.vector.scalar_tensor_tensor(
                        out=oq, in0=aq, scalar=tau, in1=eq,
                        op0=OP.is_ge, op1=OP.mult)
                    nc.sync.dma_start(
                        out=out_d[t * 128:(t + 1) * 128, q0:q0 + QW],
                        in_=oq)

    nc.compile()
    return nc


F16_MIN_NORMAL = 2.0 ** -14


def _flush16(a16):
    return np.where(np.abs(a16.astype(np.float32)) < F16_MIN_NORMAL,
                    np.float16(0), a16)


def _e4m3(a):
    import ml_dtypes
    return np.ascontiguousarray(a, dtype=np.float32).astype(
        ml_dtypes.float8_e4m3)


_NC_CACHE = {}


def _get_nc(*key):
    if key not in _NC_CACHE:
        _NC_CACHE[key] = build_nc(*key)
    return _NC_CACHE[key]


def kernel(x, W, b, k, _trace=False, _precise=True):
    """Full-input entry point: shards across 8 NeuronCores internally."""
    x = np.ascontiguousarray(np.asarray(x, dtype=np.float32))
    W = np.ascontiguousarray(np.asarray(W, dtype=np.float32))
    b = np.ascontiguousarray(np.asarray(b, dtype=np.float32)).reshape(1, -1)
    kk = int(np.asarray(k))
    Bfull, d_in = x.shape
    d_emb = W.shape[1]
    assert (Bfull, d_in, d_emb, kk) == (B, D_IN, D_EMB, K), (
        f"kernel hardcoded for {(B, D_IN, D_EMB, K)}, got "
        f"{(Bfull, d_in, d_emb, kk)}")

    rows = Bfull // N_CORES
    nc = _get_nc(rows, d_in, d_emb, kk)

    W16 = _flush16(W.astype(np.float16))
    W16f = W16.astype(np.float32)
    m8 = np.vstack([_e4m3(W * W8_SCALE), _e4m3((W - W16f) * RES_SCALE)])
    b_rep = np.ascontiguousarray(np.broadcast_to(b, (128, d_emb)))
    base = {"w16": np.ascontiguousarray(W16), "m8": m8, "b": b_rep}

    in_maps = []
    for c in range(N_CORES):
        xc = np.ascontiguousarray(x[c * rows:(c + 1) * rows])
        xct = np.ascontiguousarray(xc.T)
        x16 = _flush16(xct.astype(np.float16))
        r8 = np.vstack([_e4m3((xct - x16.astype(np.float32)) * DX_SCALE),
                        _e4m3(xct)])
        in_maps.append({"x16t": np.ascontiguousarray(x16), "r8t": r8,
                        "xn": xc, **base})
    res = run_bass_kernel_spmd(
        nc, in_maps, core_ids=list(range(N_CORES)), trace=_trace)
    out = np.concatenate([res.results[c]["out"] for c in range(N_CORES)],
                         axis=0)
    if _trace:
        return out, res
    return out

